# revision 2
# baseline (speedup 1.0000x reference)
"""Data-parallel Trainium kernel for the DMuCA (SaMCA+SeMCA) block.

Sharding (per hint): pure data parallelism — batch dim of x split across the
8 NeuronCores, all conv/BN params replicated, no cross-sample communication.
Each core runs the fused forward for its 32 samples; outputs concatenate back
to the full batch.

Hardcoded problem shape: x (256, 512, 9, 9) f32, 8 cores.
All convs are expressed as pad+slice weighted sums (9 dense fused
multiply-adds each) — these lower to plain elementwise HLO, avoiding
grouped-conv / scatter lowerings that compile poorly on the neuron backend.
"""

import numpy as np
import jax
import jax.numpy as jnp

BS, C, H, W = 256, 512, 9, 9
D = 9
SPA_HEAD = 64
HW = H * W
EPS = 1e-5
N_CORES = 8

PARAM_KEYS = (
    "alpha",
    "sa_key_w", "sa_key_bn", "sa_att_w1", "sa_att_bn", "sa_att_w2", "sa_att_b2",
    "se_key_w", "se_key_b", "se_att_w1", "se_att_bn", "se_att_w2", "se_att_b2",
    "se_val_w", "se_val_b",
)


def _bn(x, p, axis=1):
    g, b, m, v = p[0], p[1], p[2], p[3]
    sh = [1] * x.ndim
    sh[axis] = -1
    return (x - m.reshape(sh)) * (g.reshape(sh) * jax.lax.rsqrt(v.reshape(sh) + EPS)) + b.reshape(sh)


def _dw3x3(x, w):
    """Depthwise 3x3 SAME conv, per-channel kernels w (c, 3, 3)."""
    bs, c, h, wd = x.shape
    xp = jnp.pad(x, ((0, 0), (0, 0), (1, 1), (1, 1)))
    out = None
    for dy in range(3):
        for dx in range(3):
            term = xp[:, :, dy:dy + h, dx:dx + wd] * w[None, :, dy, dx, None, None]
            out = term if out is None else out + term
    return out


def _forward(x, params):
    (alpha, sa_key_w, sa_key_bn, sa_att_w1, sa_att_bn, sa_att_w2, sa_att_b2,
     se_key_w, se_key_b, se_att_w1, se_att_bn, se_att_w2, se_att_b2,
     se_val_w, se_val_b) = params
    bs, c, h, w = x.shape
    hw = h * w

    # ---------------- SeMCA (spectral attention) ----------------
    # Depthwise conv over the channel axis, one 9-tap filter per pixel:
    # k1[b, ci, p] = se_key_b[p] + sum_t se_key_w[p, t] * x[b, ci+t-4, p]
    xt = x.reshape(bs, c, hw)
    wk = se_key_w[:, 0, :]                                      # (hw, D)
    xp = jnp.pad(xt, ((0, 0), (D // 2, D // 2), (0, 0)))
    k1 = None
    for t in range(D):
        term = xp[:, t:t + c, :] * wk[None, None, :, t]
        k1 = term if k1 is None else k1 + term
    k1 = k1 + se_key_b[None, None, :]                           # (bs, c, hw)

    # kq rows: reinterpret of the (c, hw)-major flattening as (hw, c)
    rx = xt.reshape(bs, hw, c)
    rk = k1.reshape(bs, hw, c)
    a = (jnp.einsum("oi,bic->boc", se_att_w1[:, :hw], rx, preferred_element_type=jnp.float32)
         + jnp.einsum("oi,bic->boc", se_att_w1[:, hw:], rk, preferred_element_type=jnp.float32))
    a = jax.nn.relu(_bn(a, se_att_bn))                          # (bs, hw, c)
    a = jnp.einsum("oi,bic->boc", se_att_w2, a,
                   preferred_element_type=jnp.float32) + se_att_b2[None, :, None]
    a = jax.nn.softmax(a, axis=1)                               # (bs, D, c)

    v = _dw3x3(x, se_val_w[:, 0]) + se_val_b[None, :, None, None]

    # out1[b, ci] = sum_d a[b, d, ci] * v[b, ci+d-4] + k1
    vf = v.reshape(bs, c, hw)
    vp = jnp.pad(vf, ((0, 0), (D // 2, D // 2), (0, 0)))
    out1 = None
    for d in range(D):
        term = vp[:, d:d + c, :] * a[:, d, :, None]
        out1 = term if out1 is None else out1 + term
    out1 = (out1 + k1).reshape(bs, c, h, w)

    # ---------------- SaMCA (spatial attention) ----------------
    k1s = jax.nn.relu(_bn(_dw3x3(x, sa_key_w[:, 0]), sa_key_bn))

    # Grouped 1x1 conv over interleaved (k1s, x): split even/odd taps so the
    # channel interleave never materializes. Group g: 16 inputs -> 8 outputs.
    w1 = sa_att_w1[:, :, 0, 0].reshape(SPA_HEAD, C // SPA_HEAD, 2 * C // SPA_HEAD)
    w1k = w1[:, :, 0::2]                                        # (64, 8, 8) on k1s
    w1x = w1[:, :, 1::2]                                        # (64, 8, 8) on x
    k1s_g = k1s.reshape(bs, SPA_HEAD, C // SPA_HEAD, hw)
    x_g = x.reshape(bs, SPA_HEAD, C // SPA_HEAD, hw)
    a2 = (jnp.einsum("goi,bgip->bgop", w1k, k1s_g, preferred_element_type=jnp.float32)
          + jnp.einsum("goi,bgip->bgop", w1x, x_g, preferred_element_type=jnp.float32))
    a2 = jax.nn.relu(_bn(a2.reshape(bs, C, h, w), sa_att_bn))
    w2 = sa_att_w2[:, :, 0, 0]                                  # (64, 8)
    a2 = jnp.einsum("gi,bgip->bgp", w2, a2.reshape(bs, SPA_HEAD, C // SPA_HEAD, hw),
                    preferred_element_type=jnp.float32) + sa_att_b2[None, :, None]
    a2 = jax.nn.softmax(a2, axis=-1)                            # (bs, 64, hw)

    k2 = jnp.broadcast_to(a2[:, :, None, :], (bs, SPA_HEAD, C // SPA_HEAD, hw)).reshape(bs, C, hw)
    out2 = k1s + (k2 * x.reshape(bs, c, hw)).reshape(bs, c, h, w)

    return alpha[0] * out1 + (1.0 - alpha[0]) * out2


def _build():
    devs = jax.devices()
    if len(devs) >= N_CORES:
        return jax.pmap(_forward, in_axes=(0, None), devices=devs[:N_CORES]), True
    return jax.jit(_forward), False


_fn = None
_is_pmap = None


def kernel(**inputs):
    global _fn, _is_pmap
    x = np.ascontiguousarray(inputs["x"], dtype=np.float32)
    params = tuple(jnp.asarray(np.asarray(inputs[k], dtype=np.float32)) for k in PARAM_KEYS)
    if _fn is None:
        _fn, _is_pmap = _build()
    if _is_pmap:
        xs = jnp.asarray(x.reshape(N_CORES, BS // N_CORES, C, H, W))
        out = _fn(xs, params)
        out = np.asarray(jax.device_get(out)).reshape(BS, C, H, W)
    else:
        out = np.asarray(jax.device_get(_fn(jnp.asarray(x), params)))
    return out.astype(np.float32)
